# revision 45
# baseline (speedup 1.0000x reference)
"""Multi-head self-attention (B=16,T=512,C=1024,H=16) on 8 NeuronCores.

Strategy: data-parallel over batch (2 batches/core), no collectives.
All GEMMs run in fp16 (fp8 fails the accuracy gate: quantization noise on
any of the three projection paths exceeds 2e-2 max-rel-err).

v3 design (vs v1 baseline):
  - Software pipelining: projection / out-projection matmuls are woven
    between attention matmul bursts as PE "filler", so the PE never idles
    while the Scalar engine computes exp and DVE/GpSimd evacuate psum and
    apply masks.  This also keeps the PE HAM clock at 8/8.
  - All input tensors are pre-arranged on the host into the exact SBUF
    layout so every DMA moves fat contiguous per-partition chunks
    (the v1 rearranging DMAs moved 128B..2B descriptors and serialized
    the queues for ~100us).
  - exp is ONE activation per head over the whole [128, 4x512] PSUM score
    tile (stale-garbage columns are exp'd too but never read by the
    r-sliced AV matmuls).  Causal masking inside the diagonal 128x128
    blocks is one strided GpSimd multiply per head.
  - key-padding: the V evacuation scale (per-partition kpm01) zeroes
    padded key rows of v; the ones-column of v (which produces the
    softmax denominators during the AV matmul) is multiplied by kpm01.
  - softmax normalization: l rows ride along in the single per-head
    psum evacuation, are inverted with reciprocal_approx_fast, broadcast
    across partitions with one DRAM-bounce DMA per (batch, half), and
    multiplied into ao on GpSimd.
"""

import math

import numpy as np

import concourse.bass as bass
import concourse.mybir as mybir
import concourse.tile as tile
from concourse import bacc
from concourse.bass_utils import run_bass_kernel_spmd

N_CORES = 8
B, T, C = 16, 512, 1024
H = 16
DH = C // H  # 64
B_LOC = B // N_CORES  # 2
TOK = B_LOC * T  # 1024 tokens per core
P = 128
CT = C // P  # 8 contraction tiles
NR = T // P  # 4 kt blocks
F16 = mybir.dt.float16
F32 = mybir.dt.float32

DEBUG = False


def _build_nc():
    nc = bacc.Bacc("TRN2", target_bir_lowering=False, debug=False,
                   num_devices=N_CORES)

    # all host-side pre-arranged to SBUF layout (fat contiguous DMAs)
    xd = nc.dram_tensor("xd", [P, CT, TOK], F16, kind="ExternalInput").ap()
    wqkd = nc.dram_tensor("wqkd", [P, H, CT, P], F16,
                          kind="ExternalInput").ap()
    wvd = nc.dram_tensor("wvd", [P, 2, CT, 512], F16,
                         kind="ExternalInput").ap()
    wod = nc.dram_tensor("wod", [P, 2, CT, 512], F16,
                         kind="ExternalInput").ap()
    maskd = nc.dram_tensor("maskd", [P, NR, P], F16,
                           kind="ExternalInput").ap()
    kpmvd = nc.dram_tensor("kpmvd", [P, 2 * NR], F16,
                           kind="ExternalInput").ap()  # 0/1 keep, per m
    kpmsd = nc.dram_tensor("kpmsd", [P, 2 * NR], F32,
                           kind="ExternalInput").ap()  # keep as f32 scale
    biasd = nc.dram_tensor("biasd", [C], F32, kind="ExternalInput").ap()
    out = nc.dram_tensor("out", [TOK, C], F32, kind="ExternalOutput").ap()
    # DRAM bounce buffer for broadcasting 1/l across partitions
    lbounce = nc.dram_tensor("lbounce", [B_LOC, 2, H // 2, T], F16).ap()

    with tile.TileContext(nc) as tc:
        _emit(nc, tc, xd, wqkd, wvd, wod, maskd, kpmvd, kpmsd, biasd,
              out, lbounce)

    nc.compile()
    return nc


def _emit(nc, tc, xd, wqkd, wvd, wod, maskd, kpmvd, kpmsd, biasd, out,
          lbounce):
    from contextlib import ExitStack
    ctx = ExitStack()
    with ctx:
        singles = ctx.enter_context(tc.tile_pool(name="singles", bufs=1))
        ps_proj = ctx.enter_context(
            tc.tile_pool(name="ps_proj", bufs=2, space="PSUM"))
        ps_s = ctx.enter_context(
            tc.tile_pool(name="ps_s", bufs=2, space="PSUM"))
        ps_o = ctx.enter_context(
            tc.tile_pool(name="ps_o", bufs=2, space="PSUM"))
        pt_pool = ctx.enter_context(tc.tile_pool(name="pt", bufs=2))
        lin_pool = ctx.enter_context(tc.tile_pool(name="lin", bufs=2))
        lf_pool = ctx.enter_context(tc.tile_pool(name="lf", bufs=2))
        ao_st_pool = ctx.enter_context(tc.tile_pool(name="aost", bufs=2))
        y_pool = ctx.enter_context(tc.tile_pool(name="y", bufs=3))

        # --- persistent SBUF tensors ---
        x_sb = singles.tile([P, CT, TOK], F16)        # 16 KB/part
        wqk_sb = singles.tile([P, H, CT, P], F16)     # 32 KB/part
        wv_sb = singles.tile([P, 2, CT, 512], F16)    # 16 KB/part
        wo_sb = singles.tile([P, 2, CT, 512], F16)    # 16 KB/part
        qk_sb = singles.tile([P, H, TOK], F16)        # 32 KB/part
        v_sb = singles.tile([P, TOK // P, H, DH + 1], F16)  # 16.6 KB/part
        ao_b = [singles.tile([P, CT, T], F16, name=f"ao_b{b}")
                for b in range(B_LOC)]                # 2x 8 KB/part
        bias_sb = singles.tile([P, C], F32)           # 4 KB/part
        maskd_sb = singles.tile([P, NR, P], F16)      # 1 KB/part
        kpmv_sb = singles.tile([P, 2 * NR], F16)
        kpms_sb = singles.tile([P, 2 * NR], F32)

        # --- prologue DMAs (fat, contiguous per partition) ---
        def dma_wqk(j):
            nc.sync.dma_start(out=wqk_sb[:, j], in_=wqkd[:, j])

        dma_wqk(0)
        # x chunks spread across engine DMA queues so they run in parallel
        # with the weight loads instead of serializing on the sync queue;
        # first two single-k chunks let the first matmuls start earliest
        nc.scalar.dma_start(out=x_sb[:, 0:1, :], in_=xd[:, 0:1, :])
        nc.gpsimd.dma_start(out=x_sb[:, 1:2, :], in_=xd[:, 1:2, :])
        nc.scalar.dma_start(out=x_sb[:, 2:4, :], in_=xd[:, 2:4, :])
        nc.gpsimd.dma_start(out=x_sb[:, 4:6, :], in_=xd[:, 4:6, :])
        dma_wqk(8)
        nc.sync.dma_start(out=x_sb[:, 6:8, :], in_=xd[:, 6:8, :])
        nc.sync.dma_start(out=wv_sb[:, 0], in_=wvd[:, 0])
        nc.sync.dma_start(out=maskd_sb[:], in_=maskd[:])
        nc.sync.dma_start(out=kpmv_sb[:], in_=kpmvd[:])
        nc.sync.dma_start(out=kpms_sb[:], in_=kpmsd[:])

        # dummy matmuls on scratch data: warm the PE HAM clock to 8/8
        # while the first DMAs land (results are never read)
        warm_w = singles.tile([P, 512], F16)
        nc.gpsimd.memset(warm_w[:], 0.0)
        ps_warm = ps_o.tile([P, 512], F32, tag="po", name="ps_warm")
        for _ in range(10):
            nc.tensor.matmul(ps_warm[:], warm_w[:, 0:P], warm_w[:],
                             start=True, stop=True, skip_group_check=True)

        # ones-column of v = kpm01 (memset + broadcast multiply)
        ones_l = singles.tile([H // 2, T], F16)  # numerator for 1/l divide
        nc.gpsimd.memset(ones_l[:], 1.0)
        ones_col = v_sb[:, :, :, DH:DH + 1]
        nc.gpsimd.memset(ones_col, 1.0)
        # preload the Exp activation table off the critical path
        exp_warm = singles.tile([1, 1], F16)
        nc.scalar.activation(out=exp_warm[:], in_=ones_l[0:1, 0:1],
                             func=mybir.ActivationFunctionType.Exp)
        kpm_b = bass.AP(tensor=kpmv_sb.tensor, offset=kpmv_sb[:].offset,
                        ap=[kpmv_sb[:].ap[0], [1, 2 * NR], [0, H]])
        oc3 = bass.AP(tensor=v_sb.tensor, offset=ones_col.offset,
                      ap=[ones_col.ap[0], [H * (DH + 1), 2 * NR],
                          [DH + 1, H]])
        nc.vector.tensor_mul(out=oc3, in0=oc3, in1=kpm_b)

        # --- emitters ---
        def qk_j(j):
            for tt in range(2):
                ps = ps_proj.tile([P, 512], F32, tag="ps_proj",
                                  name=f"ps_qk_{j}_{tt}")
                for k in range(CT):
                    nc.tensor.matmul(
                        ps[:], wqk_sb[:, j, k, :],
                        x_sb[:, k, tt * 512:(tt + 1) * 512],
                        start=(k == 0), stop=(k == CT - 1))
                nc.vector.tensor_copy(
                    out=qk_sb[:, j, tt * 512:(tt + 1) * 512], in_=ps[:])

        def v_nm(n, m):
            ps = ps_proj.tile([P, 512], F32, tag="ps_proj",
                              name=f"ps_v_{n}_{m}")
            for k in range(CT):
                nc.tensor.matmul(
                    ps[:], x_sb[:, k, m * P:(m + 1) * P], wv_sb[:, n, k, :],
                    start=(k == 0), stop=(k == CT - 1))
            nc.vector.tensor_scalar_mul(
                out=v_sb[:, m, 8 * n:8 * n + 8, 0:DH],
                in0=ps[:].rearrange("p (h d) -> p h d", d=DH),
                scalar1=kpms_sb[:, m:m + 1])

        def scores_pair_half(b, hA, hB, pTA, pTB, half):
            """Score matmuls for r=2*half,2*half+1 of BOTH heads of a pair,
            interleaved A/B.  Head A lives in partitions 0-63, head B in
            64-127, so adjacent A/B matmuls land in different PE row groups
            and execute concurrently (~2x).  Then exp+diag-mask per head."""
            sts = []
            for h, pT in ((hA, pTA), (hB, pTB)):
                sts.append(ps_s.tile([P, 2, 512], F32, tag="sT2",
                                     name=f"sT2_{b}_{h}_{half}"))
            for (h, pT), sT2 in zip(((hA, pTA), (hB, pTB)), sts):
                jq, jk, dlo = h // 2, 8 + h // 2, DH * (h % 2)
                for rr in range(2):
                    r = 2 * half + rr
                    kT = qk_sb[dlo:dlo + DH, jk,
                               b * T + r * P: b * T + (r + 1) * P]
                    qTr = qk_sb[dlo:dlo + DH, jq,
                                b * T + r * P:(b + 1) * T]
                    nc.tensor.matmul(sT2[:, rr, r * P:], kT, qTr,
                                     start=True, stop=True)
            for (h, pT), sT2 in zip(((hA, pTA), (hB, pTB)), sts):
                nc.scalar.activation(
                    out=pT[:, 2 * half:2 * half + 2, :], in_=sT2[:],
                    func=mybir.ActivationFunctionType.Exp)
                base = pT[:, 2 * half, 2 * half * P:(2 * half + 1) * P]
                diag = bass.AP(tensor=base.tensor, offset=base.offset,
                               ap=[base.ap[0], [512 + P, 2], [1, P]])
                nc.gpsimd.tensor_mul(
                    out=diag, in0=diag,
                    in1=maskd_sb[:, 2 * half:2 * half + 2, :])

        def av(b, h, pT):
            po = ps_o.tile([P, 512], F32, tag="po", name=f"po_{b}_{h}")
            for r in range(NR):
                nc.tensor.matmul(po[0:DH + 1, r * P:],
                                 v_sb[:, b * NR + r, h, :],
                                 pT[:, r, r * P:],
                                 start=(r == 0), stop=(r == NR - 1))
            return po

        lpart = {}

        def finish_head(b, h, po, lpart_t):
            # one evacuation of attention-out rows + the l row (row DH),
            # then SBUF->SBUF DMAs place them (DMA can shift partitions)
            ao_st = ao_st_pool.tile([DH + 1, 512], F16, tag="ao_st")
            nc.vector.tensor_copy(out=ao_st[:], in_=po[0:DH + 1, :])
            k, dlo = h // 2, DH * (h % 2)
            nc.sync.dma_start(out=ao_b[b][dlo:dlo + DH, k, :],
                              in_=ao_st[0:DH, :])
            nc.sync.dma_start(out=lpart_t[h % 4:h % 4 + 1, :],
                              in_=ao_st[DH:DH + 1, :])

        def attn_pair(b, p, filler1, filler2):
            hA, hB = 2 * p, 2 * p + 1
            key = (b, p // 2)
            if key not in lpart:
                lpart[key] = lin_pool.tile([4, T], F16, tag="lpart",
                                           name=f"lpart_{key[0]}_{key[1]}")
            lp = lpart[key]
            fillers = list(filler1) + list(filler2)
            fa, fb, fc = fillers[0::3], fillers[1::3], fillers[2::3]
            pA = pt_pool.tile([P, NR, 512], F16, tag="pT",
                              name=f"pT_{b}_{hA}")
            pB = pt_pool.tile([P, NR, 512], F16, tag="pT",
                              name=f"pT_{b}_{hB}")
            scores_pair_half(b, hA, hB, pA, pB, 0)
            for f in fa:
                f()
            scores_pair_half(b, hA, hB, pA, pB, 1)
            for f in fb:
                f()
            poA = av(b, hA, pA)
            finish_head(b, hA, poA, lp)
            for f in fc:
                f()
            poB = av(b, hB, pB)
            finish_head(b, hB, poB, lp)

        def norm_quarter(b, qi):
            """Invert l for 4 heads (2 k-tiles) and normalize their ao.
            Quarter granularity keeps the chain that gates the final
            out-projections short."""
            half, qq = qi // 2, qi % 2
            lp = lpart[(b, qi)]
            tag = f"{b}_{qi}"
            lp32 = lin_pool.tile([4, T], F32, tag="lp32",
                                 name=f"lp32_{tag}")
            nc.vector.tensor_copy(out=lp32[:], in_=lp[:])
            linv = lin_pool.tile([4, T], F32, tag="linv",
                                 name=f"linv_{tag}")
            nc.vector.reciprocal_approx_fast(out=linv[:], in_=lp32[:])
            linv16 = lin_pool.tile([4, T], F16, tag="linv16",
                                   name=f"linv16_{tag}")
            nc.vector.tensor_copy(out=linv16[:], in_=linv[:])
            nc.sync.dma_start(out=lbounce[b, half, 4 * qq:4 * qq + 4, :],
                              in_=linv16[:])
            lf = lf_pool.tile([P, 2, 512], F16, tag="lf",
                              name=f"lf_{tag}")
            boff = (lbounce.offset + (b * 2 + half) * (H // 2) * T
                    + 4 * qq * T)
            for ph in range(2):  # even heads -> parts 0-63, odd -> 64-127
                src = bass.AP(tensor=lbounce.tensor, offset=boff + ph * T,
                              ap=[[0, DH], [2 * T, 2], [1, T]])
                nc.sync.dma_start(out=lf[ph * DH:(ph + 1) * DH, :, :],
                                  in_=src)
            ks = slice(2 * qi, 2 * qi + 2)
            # late norms gate the final out-projections: run them on DVE
            # (f16 4x mode, ~5x faster than GpSimd here)
            eng = nc.vector if b == B_LOC - 1 and qi >= 2 else nc.gpsimd
            eng.tensor_mul(out=ao_b[b][:, ks, :],
                           in0=ao_b[b][:, ks, :], in1=lf[:])

        def yproj(b, i):
            n, m = i // NR, i % NR
            ps = ps_proj.tile([P, 512], F32, tag="ps_proj",
                              name=f"ps_y_{b}_{n}_{m}")
            for kx, k in enumerate(range(CT)):
                nc.tensor.matmul(
                    ps[:], ao_b[b][:, k, m * P:(m + 1) * P],
                    wo_sb[:, n, k, :],
                    start=(kx == 0), stop=(kx == CT - 1))
            y = y_pool.tile([P, 512], F32, tag="y")
            nc.vector.tensor_add(out=y[:], in0=ps[:],
                                 in1=bias_sb[:, n * 512:(n + 1) * 512])
            out_eng = [nc.sync, nc.scalar, nc.gpsimd][(b * 2 * NR + i) % 3]
            out_eng.dma_start(
                out=out[b * T + m * P: b * T + (m + 1) * P,
                        n * 512:(n + 1) * 512],
                in_=y[:])

        def dma_wo_bias():
            bias_bcast = bass.AP(tensor=biasd.tensor, offset=biasd.offset,
                                 ap=[[0, P], *biasd.ap])
            nc.gpsimd.dma_start(out=bias_sb[:], in_=bias_bcast)
            for n in range(2):
                nc.sync.dma_start(out=wo_sb[:, n], in_=wod[:, n])

        # --- emission schedule ---
        qk_j(0)
        qk_j(8)
        dma_wqk(1)
        dma_wqk(9)
        for m in range(4):
            v_nm(0, m)

        fill_b0 = {
            0: ([lambda: dma_wqk(2), lambda: dma_wqk(10),
                 lambda: qk_j(1), lambda: qk_j(9)], [lambda: v_nm(1, 0)]),
            1: ([lambda: dma_wqk(3), lambda: dma_wqk(11),
                 lambda: qk_j(2), lambda: qk_j(10)], [lambda: v_nm(1, 1)]),
            2: ([lambda: dma_wqk(4), lambda: dma_wqk(12),
                 lambda: qk_j(3), lambda: qk_j(11)], [lambda: v_nm(1, 2)]),
            3: ([lambda: dma_wqk(5), lambda: dma_wqk(13),
                 lambda: qk_j(4), lambda: qk_j(12)],
                [lambda: v_nm(1, 3), dma_wo_bias]),
            4: ([lambda: dma_wqk(6), lambda: dma_wqk(14),
                 lambda: qk_j(5), lambda: qk_j(13)], [lambda: v_nm(0, 4)]),
            5: ([lambda: dma_wqk(7), lambda: dma_wqk(15),
                 lambda: qk_j(6), lambda: qk_j(14)], [lambda: v_nm(0, 5)]),
            6: ([lambda: qk_j(7), lambda: qk_j(15)], [lambda: v_nm(0, 6)]),
            7: ([lambda: v_nm(0, 7)], [lambda: v_nm(1, 4)]),
        }
        # wv n=1 needed from b0 pair 0's filler v_nm(1,0)
        nc.sync.dma_start(out=wv_sb[:, 1], in_=wvd[:, 1])
        for p in range(8):
            f1, f2 = fill_b0[p]
            attn_pair(0, p, f1, f2)
            if p % 2 == 1:
                norm_quarter(0, p // 2)

        # keep yproj(0, 5..7) in reserve: they cover the PE while the last
        # norm_half(1,1) chain resolves, instead of a dead stall
        fill_b1 = {
            0: ([lambda: v_nm(1, 5)], [lambda: v_nm(1, 6)]),
            1: ([lambda: v_nm(1, 7)], [lambda: yproj(0, 0)]),
            2: ([lambda: yproj(0, 1)], []),
            3: ([lambda: yproj(0, 2)], []),
            4: ([lambda: yproj(0, 3)], [lambda: yproj(0, 4)]),
            5: ([lambda: yproj(0, 5)], []),
            6: ([lambda: yproj(0, 6)], []),
            7: ([lambda: yproj(0, 7)], []),
        }
        for p in range(8):
            f1, f2 = fill_b1[p]
            attn_pair(1, p, f1, f2)
            if p % 2 == 1:
                norm_quarter(1, p // 2)  # last call only gates k-tiles 6,7

        for i in range(2 * NR):
            yproj(1, i)

        if DEBUG:
            dq = nc.dram_tensor("dbg_qk", [P, H, TOK], F16,
                                kind="ExternalOutput").ap()
            dv = nc.dram_tensor("dbg_v", [P, TOK // P, H, DH + 1], F16,
                                kind="ExternalOutput").ap()
            da = nc.dram_tensor("dbg_ao", [B_LOC, P, CT, T], F16,
                                kind="ExternalOutput").ap()
            nc.sync.dma_start(out=dq[:], in_=qk_sb[:])
            nc.sync.dma_start(out=dv[:], in_=v_sb[:])
            for b in range(B_LOC):
                nc.sync.dma_start(out=da[b], in_=ao_b[b][:])


_NC_CACHE = None


def _get_nc():
    global _NC_CACHE
    if _NC_CACHE is None:
        _NC_CACHE = _build_nc()
    return _NC_CACHE


def _prep_core_inputs(x, mask, key_padding_mask, w_qkv, w_out, b_out):
    """Host-side sharding + layout prep. Returns list of per-core in_maps."""
    x = np.asarray(x, dtype=np.float32)
    mask = np.asarray(mask)
    kpm = np.asarray(key_padding_mask)
    w_qkv = np.asarray(w_qkv, dtype=np.float32)
    w_out = np.asarray(w_out, dtype=np.float32)
    b_out = np.asarray(b_out, dtype=np.float32)

    FQK = 2 * C
    wqkT = w_qkv[:FQK].T.copy()  # [C, 2C]
    wqkT[:, :C] *= 1.0 / math.sqrt(DH)  # fold 1/sqrt(dh) into Q weights
    # [P, j, k, f]
    wqkd = np.ascontiguousarray(
        wqkT.astype(np.float16).reshape(CT, P, H, P).transpose(1, 2, 0, 3))
    wvd = np.ascontiguousarray(
        w_qkv[FQK:].T.astype(np.float16).reshape(CT, P, 2, 512)
        .transpose(1, 2, 0, 3))
    wod = np.ascontiguousarray(
        w_out.T.astype(np.float16).reshape(CT, P, 2, 512)
        .transpose(1, 2, 0, 3))

    exp_tril = np.tril(np.ones((T, T), dtype=mask.dtype))
    assert np.array_equal(mask, exp_tril), "kernel assumes causal tril mask"
    maskTf = mask.T.astype(np.float16)  # [kt, qt]
    maskd = np.ascontiguousarray(
        np.stack([maskTf[r * P:(r + 1) * P, r * P:(r + 1) * P]
                  for r in range(NR)]).transpose(1, 0, 2))  # [P, NR, P]

    in_maps = []
    for i in range(N_CORES):
        xs = x[i * B_LOC:(i + 1) * B_LOC]      # [B_LOC, T, C]
        xdd = np.ascontiguousarray(
            xs.reshape(TOK, C).T.astype(np.float16)
            .reshape(CT, P, TOK).transpose(1, 0, 2))  # [P, k, t]
        keep = (~kpm[i * B_LOC:(i + 1) * B_LOC]).astype(np.float32)
        keep_pm = keep.reshape(2 * NR, P).T  # [P, m]
        in_maps.append({
            "xd": xdd,
            "wqkd": wqkd,
            "wvd": wvd,
            "wod": wod,
            "maskd": maskd,
            "kpmvd": np.ascontiguousarray(keep_pm.astype(np.float16)),
            "kpmsd": np.ascontiguousarray(keep_pm.astype(np.float32)),
            "biasd": b_out,
        })
    return in_maps


def kernel(x, mask, key_padding_mask, w_qkv, w_out, b_out, _trace=False,
           _tmpdir=None):
    nc = _get_nc()
    in_maps = _prep_core_inputs(x, mask, key_padding_mask, w_qkv, w_out,
                                b_out)
    res = run_bass_kernel_spmd(nc, in_maps, list(range(N_CORES)),
                               trace=_trace, tmpdir=_tmpdir)
    outs = [res.results[i]["out"].reshape(B_LOC, T, C)
            for i in range(N_CORES)]
    full = np.concatenate(outs, axis=0).astype(np.float32)
    kernel._last_exec_time_ns = res.exec_time_ns
    return full
